# revision 17
# baseline (speedup 1.0000x reference)
"""DMTet marching-tetrahedra geometry kernel for 8 Trainium2 NeuronCores.

Pipeline (matches reference semantics exactly):
  device A (tet-sharded, 8 cores): occ4 -> tetindex, 6 sorted edge pairs/tet
  host:    valid filter, 36-bit edge-key sort/dedup (rank order), crossing
           mask, cumsum rank mapping, record gathers for interpolation
  device B (edge-sharded, 8 cores): per-edge linear interpolation of the
           crossing point (verts) + uv grid generation
  host:    triangle-table face assembly, uv_idx arithmetic, output assembly
"""

import os
import numpy as np

import concourse.bass as bass
import concourse.tile as tile
from concourse import mybir
from concourse.bass_utils import run_bass_kernel_spmd

NV = 200_000
NT = 800_000
NCORES = 8
P = 128

TRIANGLE_TABLE = np.array([
    [-1, -1, -1, -1, -1, -1], [1, 0, 2, -1, -1, -1], [4, 0, 3, -1, -1, -1],
    [1, 4, 2, 1, 3, 4], [3, 1, 5, -1, -1, -1], [2, 3, 0, 2, 5, 3],
    [1, 4, 0, 1, 5, 4], [4, 2, 5, -1, -1, -1], [4, 5, 2, -1, -1, -1],
    [4, 1, 0, 4, 5, 1], [3, 2, 0, 3, 5, 2], [1, 3, 5, -1, -1, -1],
    [4, 1, 2, 4, 3, 1], [3, 0, 4, -1, -1, -1], [2, 0, 1, -1, -1, -1],
    [-1, -1, -1, -1, -1, -1]], dtype=np.int32)
NUM_TRI_TABLE = np.array([0, 1, 1, 2, 1, 2, 2, 1, 1, 2, 2, 1, 2, 1, 1, 0],
                         dtype=np.int32)
EDGE_PAIRS = [(0, 1), (0, 2), (0, 3), (1, 2), (1, 3), (2, 3)]

# Per-core tet shard: NT/8 = 100000, padded to a multiple of 128.
NTPC = 100_000
TETK = (NTPC + P - 1) // P          # 782 -> padded count 100096
NTPC_PAD = P * TETK

_TRACE = bool(os.environ.get("DMTET_KERNEL_TRACE"))
LAST_EXEC_NS = {}                    # program name -> exec_time_ns (when tracing)

# ---------------------------------------------------------------------------
# walrus in this toolchain accepts at most ONE sync wait per instruction.
# Split multi-wait instructions: excess waits move onto injected
# wait-only InstEventSemaphore instructions placed just before, same engine.
# ---------------------------------------------------------------------------
_MAX_WAITS = 1
_wsplit_uid = [0]


def _legalize_waits(nc):
    for f in nc.m.functions:
        for bb in f.blocks:
            insts = bb.instructions
            out = []
            changed = False
            for inst in insts:
                si = inst.sync_info
                w = list(si.on_wait) if (si is not None and si.on_wait) else []
                if len(w) > _MAX_WAITS:
                    changed = True
                    extra, keep = w[:-_MAX_WAITS], w[-_MAX_WAITS:]
                    for k in range(0, len(extra), _MAX_WAITS):
                        _wsplit_uid[0] += 1
                        out.append(mybir.InstEventSemaphore(
                            name=f"WSPLIT-{_wsplit_uid[0]}",
                            engine=inst.engine,
                            ins=[], outs=[],
                            sync_info=mybir.SyncInfo(
                                on_wait=extra[k:k + _MAX_WAITS], on_update=[]),
                        ))
                    si.on_wait = keep
                out.append(inst)
            if changed:
                bb.instructions = out


def _maybe_install_trace_hook():
    """Register the axon NTFF profile hook if the image's antenv lacks it."""
    if not _TRACE:
        return
    try:
        import antenv.axon_hooks  # noqa: F401
        return
    except ImportError:
        pass
    try:
        import sys
        import types
        import trn_agent_boot.trn_boot as tb
        hook = tb._ntff_profile_via_ctypes('/opt/axon/libaxon_pjrt.so')
        mod = types.ModuleType('antenv.axon_hooks')
        mod.get_axon_ntff_profile_hook = lambda: hook
        mod.set_axon_ntff_profile_hook = lambda h: None
        import antenv
        antenv.axon_hooks = mod
        sys.modules['antenv.axon_hooks'] = mod
    except Exception:
        pass


# ---------------------------------------------------------------------------
# Device program A: per-tet stage.
#   in : tet4 [NTPC_PAD, 4] i32, occ4 [NTPC_PAD, 4] i32 (0/1)
#   out: ea6/eb6 [NTPC_PAD, 6] i32 (sorted edge endpoint pairs),
#        tindex [NTPC_PAD] i32 (4-bit marching-tets code)
# ---------------------------------------------------------------------------
_prog_cache = {}


_BUILDERS_SRC = r'''
def _build_tet_program():
    nc = bass.Bass("TRN2", target_bir_lowering=False, debug=False,
                   num_devices=NCORES, disable_frame_to_traceback=True)
    i32 = mybir.dt.int32
    tet4 = nc.dram_tensor("tet4", [NTPC_PAD, 4], i32, kind="ExternalInput")
    occ4 = nc.dram_tensor("occ4", [NTPC_PAD, 4], mybir.dt.uint8,
                          kind="ExternalInput")
    ea6 = nc.dram_tensor("ea6", [NTPC_PAD, 6], i32, kind="ExternalOutput")
    eb6 = nc.dram_tensor("eb6", [NTPC_PAD, 6], i32, kind="ExternalOutput")
    tindex = nc.dram_tensor("tindex", [NTPC_PAD], i32, kind="ExternalOutput")

    K = TETK
    tet_v = tet4[:].rearrange("(p k) j -> p (k j)", p=P)
    occ_v = occ4[:].rearrange("(p k) j -> p (k j)", p=P)
    ea_v = ea6[:].rearrange("(p k) j -> p (k j)", p=P)
    eb_v = eb6[:].rearrange("(p k) j -> p (k j)", p=P)
    ti_v = tindex[:].rearrange("(p k) -> p k", p=P)

    with tile.TileContext(nc) as tc:
        with tc.tile_pool(name="sb", bufs=1) as pool:
            t_t = pool.tile([P, K, 4], i32)
            o_t = pool.tile([P, K, 4], i32)
            nc.sync.dma_start(t_t[:].rearrange("p k j -> p (k j)"), tet_v)
            # uint8 -> int32 cast during SWDGE DMA
            nc.gpsimd.dma_start(o_t[:].rearrange("p k j -> p (k j)"), occ_v)

            # tetindex = o0 + 2 o1 + 4 o2 + 8 o3
            ti_t = pool.tile([P, K], i32)
            acc = pool.tile([P, K], i32)
            nc.vector.tensor_copy(ti_t[:], o_t[:, :, 0])
            for j, w in ((1, 2), (2, 4), (3, 8)):
                nc.vector.tensor_scalar(out=acc[:], in0=o_t[:, :, j],
                                        scalar1=w, scalar2=None,
                                        op0=mybir.AluOpType.mult)
                nc.vector.tensor_tensor(out=ti_t[:], in0=ti_t[:], in1=acc[:],
                                        op=mybir.AluOpType.add)
            nc.sync.dma_start(ti_v, ti_t[:])

            # 6 sorted vertex pairs per tet
            ea_t = pool.tile([P, K, 6], i32)
            eb_t = pool.tile([P, K, 6], i32)
            for e, (u, v) in enumerate(EDGE_PAIRS):
                nc.vector.tensor_tensor(out=ea_t[:, :, e], in0=t_t[:, :, u],
                                        in1=t_t[:, :, v],
                                        op=mybir.AluOpType.min)
                nc.vector.tensor_tensor(out=eb_t[:, :, e], in0=t_t[:, :, u],
                                        in1=t_t[:, :, v],
                                        op=mybir.AluOpType.max)
            nc.sync.dma_start(ea_v, ea_t[:].rearrange("p k j -> p (k j)"))
            nc.sync.dma_start(eb_v, eb_t[:].rearrange("p k j -> p (k j)"))
    return nc


# ---------------------------------------------------------------------------
# Device program B: per-edge interpolation + uv grid.
#   in : recA/recB [EPC, 4] f32 (x, y, z, sdf) for edge endpoints a/b,
#        linj [UVN] f32, lini [P] f32 (this core's 128 uv grid rows)
#   out: verts [EPC, 3] f32, uvblk [P, UVN, 8] f32
# ---------------------------------------------------------------------------
UVN = 895          # ceil(sqrt((2*NT+1)//2))
UVPAD_CONST = np.float32(0.9 / UVN)


def _build_interp_program(ek):
    """ek: free-dim length per partition of the edge shard (EPC = 128*ek)."""
    nc = bass.Bass("TRN2", target_bir_lowering=False, debug=False,
                   num_devices=NCORES, disable_frame_to_traceback=True)
    f32 = mybir.dt.float32
    epc = P * ek
    recA = nc.dram_tensor("recA", [epc, 4], f32, kind="ExternalInput")
    recB = nc.dram_tensor("recB", [epc, 4], f32, kind="ExternalInput")
    linj = nc.dram_tensor("linj", [P, UVN], f32, kind="ExternalInput")
    lini = nc.dram_tensor("lini", [P], f32, kind="ExternalInput")
    verts = nc.dram_tensor("verts", [epc, 3], f32, kind="ExternalOutput")
    uvblk = nc.dram_tensor("uvblk", [P, UVN, 8], f32, kind="ExternalOutput")

    EKB = 512
    nblk = (ek + EKB - 1) // EKB
    with tile.TileContext(nc) as tc:
        with tc.tile_pool(name="io", bufs=3) as pio, \
             tc.tile_pool(name="tmp", bufs=2) as ptmp:
            # ---- uv grid block ----
            lj = pio.tile([P, UVN], f32, tag="lj")
            nc.sync.dma_start(lj[:], linj[:, :])
            li = pio.tile([P, 1], f32, tag="li")
            nc.sync.dma_start(li[:], lini[:, None])
            uv = pio.tile([P, UVN, 8], f32, tag="uv")
            ljp = ptmp.tile([P, UVN], f32, tag="ljp")
            nc.vector.tensor_scalar(out=ljp[:], in0=lj[:],
                                    scalar1=float(UVPAD_CONST), scalar2=None,
                                    op0=mybir.AluOpType.add)
            lib = ptmp.tile([P, UVN], f32, tag="lib")
            nc.vector.tensor_scalar(out=lib[:], in0=li[:].to_broadcast([P, UVN]),
                                    scalar1=0.0, scalar2=None,
                                    op0=mybir.AluOpType.add)
            libp = ptmp.tile([P, UVN], f32, tag="libp")
            nc.vector.tensor_scalar(out=libp[:], in0=lib[:],
                                    scalar1=float(UVPAD_CONST), scalar2=None,
                                    op0=mybir.AluOpType.add)
            for c, src in ((0, lj), (1, lib), (2, ljp), (3, lib),
                           (4, ljp), (5, libp), (6, lj), (7, libp)):
                nc.vector.tensor_copy(uv[:, :, c], src[:])
            nc.sync.dma_start(uvblk[:, :, :].rearrange("p k j -> p (k j)"),
                              uv[:].rearrange("p k j -> p (k j)"))

            # ---- interpolation blocks ----
            recA_v = recA[:].rearrange("(p k) j -> p k j", p=P)
            recB_v = recB[:].rearrange("(p k) j -> p k j", p=P)
            verts_v = verts[:].rearrange("(p k) j -> p k j", p=P)
            for b in range(nblk):
                k0 = b * EKB
                kb = min(EKB, ek - k0)
                a_t = pio.tile([P, EKB, 4], f32, tag="a")
                b_t = pio.tile([P, EKB, 4], f32, tag="b")
                nc.sync.dma_start(
                    a_t[:, :kb, :].rearrange("p k j -> p (k j)"),
                    recA_v[:, k0:k0 + kb, :].rearrange("p k j -> p (k j)"))
                nc.sync.dma_start(
                    b_t[:, :kb, :].rearrange("p k j -> p (k j)"),
                    recB_v[:, k0:k0 + kb, :].rearrange("p k j -> p (k j)"))
                d_t = ptmp.tile([P, EKB], f32, tag="d")
                nc.vector.tensor_tensor(out=d_t[:, :kb], in0=a_t[:, :kb, 3],
                                        in1=b_t[:, :kb, 3],
                                        op=mybir.AluOpType.subtract)
                # r = 1/d via hw reciprocal + one Newton step (~1e-7 rel)
                r_t = ptmp.tile([P, EKB], f32, tag="r")
                nc.vector.reciprocal(r_t[:, :kb], d_t[:, :kb])
                nt_t = ptmp.tile([P, EKB], f32, tag="nt")
                nc.vector.tensor_tensor(out=nt_t[:, :kb], in0=d_t[:, :kb],
                                        in1=r_t[:, :kb],
                                        op=mybir.AluOpType.mult)
                nc.vector.tensor_scalar(out=nt_t[:, :kb], in0=nt_t[:, :kb],
                                        scalar1=-1.0, scalar2=2.0,
                                        op0=mybir.AluOpType.mult,
                                        op1=mybir.AluOpType.add)
                nc.vector.tensor_tensor(out=r_t[:, :kb], in0=r_t[:, :kb],
                                        in1=nt_t[:, :kb],
                                        op=mybir.AluOpType.mult)
                # wa = (-sb) * r ; wb = sa * r
                wa_t = ptmp.tile([P, EKB], f32, tag="wa")
                nc.vector.tensor_scalar(out=wa_t[:, :kb], in0=b_t[:, :kb, 3],
                                        scalar1=-1.0, scalar2=None,
                                        op0=mybir.AluOpType.mult)
                nc.vector.tensor_tensor(out=wa_t[:, :kb], in0=wa_t[:, :kb],
                                        in1=r_t[:, :kb],
                                        op=mybir.AluOpType.mult)
                wb_t = ptmp.tile([P, EKB], f32, tag="wb")
                nc.vector.tensor_tensor(out=wb_t[:, :kb], in0=a_t[:, :kb, 3],
                                        in1=r_t[:, :kb],
                                        op=mybir.AluOpType.mult)
                v_t = pio.tile([P, EKB, 3], f32, tag="v")
                m_t = ptmp.tile([P, EKB], f32, tag="m")
                for c in range(3):
                    nc.vector.tensor_tensor(out=m_t[:, :kb],
                                            in0=a_t[:, :kb, c],
                                            in1=wa_t[:, :kb],
                                            op=mybir.AluOpType.mult)
                    nc.vector.tensor_tensor(out=v_t[:, :kb, c],
                                            in0=b_t[:, :kb, c],
                                            in1=wb_t[:, :kb],
                                            op=mybir.AluOpType.mult)
                    nc.vector.tensor_tensor(out=v_t[:, :kb, c],
                                            in0=v_t[:, :kb, c],
                                            in1=m_t[:, :kb],
                                            op=mybir.AluOpType.add)
                nc.sync.dma_start(
                    verts_v[:, k0:k0 + kb, :].rearrange("p k j -> p (k j)"),
                    v_t[:, :kb, :].rearrange("p k j -> p (k j)"))
    return nc



'''

# exec with a fixed pseudo-filename so the BIR debug table (and hence the
# NEFF cache key) is independent of the directory kernel.py runs from
exec(compile(_BUILDERS_SRC, "dmtet_builders.py", "exec"), globals())

# Normalize instruction/allocation debug info the same way: caller frames
# otherwise leak harness file paths into the serialized BIR.
_orig_get_debug_info = bass.Bass.get_debug_info


def _stable_get_debug_info(self):
    d = _orig_get_debug_info(self)
    return mybir.OpDebugInfo(
        op_name=d.op_name, tensorizer_id=d.tensorizer_id,
        filename=d.filename if d.filename == "dmtet_builders.py"
        else "kernel.py",
        lineno=d.lineno, bass_funcname=d.bass_funcname,
        kernel_name=d.kernel_name, ant_traceback="",
        ant_layer=d.ant_layer, ant_annotation=d.ant_annotation)


bass.Bass.get_debug_info = _stable_get_debug_info


_exec_cache = {}


def _cached_spmd_run(nc, in_maps):
    """Like bass2jax.run_bass_via_pjrt but with the jitted executable cached
    across kernel() calls (a fresh jit per call costs seconds in NEFF
    load/dispatch)."""
    import jax
    from jax.sharding import Mesh, PartitionSpec
    try:
        from jax.experimental.shard_map import shard_map
    except ImportError:
        from jax.shard_map import shard_map
    from concourse import bass2jax

    key = id(nc)
    if key not in _exec_cache:
        bass2jax.install_neuronx_cc_hook()
        partition_name = (nc.partition_id_tensor.name
                          if nc.partition_id_tensor else None)
        in_names, out_names, out_avals = [], [], []
        for alloc in nc.m.functions[0].allocations:
            if not isinstance(alloc, mybir.MemoryLocationSet):
                continue
            name = alloc.memorylocations[0].name
            if alloc.kind == "ExternalInput":
                if name != partition_name:
                    in_names.append(name)
            elif alloc.kind == "ExternalOutput":
                out_names.append(name)
                out_avals.append(jax.core.ShapedArray(
                    tuple(alloc.tensor_shape), mybir.dt.np(alloc.dtype)))
        n_params = len(in_names)
        all_names = list(in_names) + list(out_names)
        if partition_name is not None:
            all_names.append(partition_name)

        def _body(*args):
            operands = list(args)
            if partition_name is not None:
                operands.append(bass2jax.partition_id_tensor())
            outs = bass2jax._bass_exec_p.bind(
                *operands,
                out_avals=tuple(out_avals),
                in_names=tuple(all_names),
                out_names=tuple(out_names),
                lowering_input_output_aliases=(),
                sim_require_finite=True,
                sim_require_nnan=True,
                nc=nc,
            )
            return tuple(outs)

        devices = jax.devices()[:NCORES]
        mesh = Mesh(np.asarray(devices), ("core",))
        n_outs = len(out_names)
        sharded = jax.jit(
            shard_map(_body, mesh=mesh,
                      in_specs=(PartitionSpec("core"),) * (n_params + n_outs),
                      out_specs=(PartitionSpec("core"),) * n_outs,
                      check_rep=False),
            donate_argnums=tuple(range(n_params, n_params + n_outs)),
            keep_unused=True)
        _exec_cache[key] = (sharded, in_names, out_names, out_avals, n_params)

    sharded, in_names, out_names, out_avals, n_params = _exec_cache[key]
    concat_in = [np.concatenate([np.asarray(m[name]) for m in in_maps], axis=0)
                 for name in in_names]
    concat_zeros = [np.zeros((NCORES * a.shape[0], *a.shape[1:]), a.dtype)
                    for a in out_avals]
    out_arrs = sharded(*concat_in, *concat_zeros)
    return [{name: np.asarray(out_arrs[i]).reshape(
                NCORES, *out_avals[i].shape)[c]
             for i, name in enumerate(out_names)}
            for c in range(NCORES)]


def _run(nc, in_maps, name):
    if not getattr(nc, "_waits_legalized", False):
        _legalize_waits(nc)
        nc._waits_legalized = True
    if _TRACE:
        _maybe_install_trace_hook()
        res = run_bass_kernel_spmd(nc, in_maps, core_ids=list(range(NCORES)),
                                   trace=True)
        if res.exec_time_ns is not None:
            LAST_EXEC_NS[name] = res.exec_time_ns
        return res.results
    return _cached_spmd_run(nc, in_maps)


# ---------------------------------------------------------------------------
# Host orchestration
# ---------------------------------------------------------------------------

def kernel(pos_nx3, sdf_n, tet_fx4):
    pos = np.ascontiguousarray(np.asarray(pos_nx3, dtype=np.float32))
    sdf = np.ascontiguousarray(np.asarray(sdf_n, dtype=np.float32))
    tet = np.ascontiguousarray(np.asarray(tet_fx4, dtype=np.int32))

    occ = sdf > 0
    occ4 = occ[tet].astype(np.uint8)                     # [NT, 4]

    # ---- device A: tetindex + sorted edge pairs ----
    if "tet" not in _prog_cache:
        _prog_cache["tet"] = _build_tet_program()
    nc_a = _prog_cache["tet"]
    pad = NTPC_PAD * NCORES - NT
    tet_p = np.concatenate([tet, np.zeros((pad, 4), np.int32)], 0)
    occ4_p = np.concatenate([occ4, np.zeros((pad, 4), np.uint8)], 0)
    in_maps = [{"tet4": tet_p[c * NTPC_PAD:(c + 1) * NTPC_PAD],
                "occ4": occ4_p[c * NTPC_PAD:(c + 1) * NTPC_PAD]}
               for c in range(NCORES)]
    res_a = _run(nc_a, in_maps, "tet")
    ea6 = np.concatenate([r["ea6"] for r in res_a], 0)[:NT]
    eb6 = np.concatenate([r["eb6"] for r in res_a], 0)[:NT]
    ti = np.concatenate([r["tindex"] for r in res_a], 0)[:NT]

    # ---- host: valid filter + sort/dedup + rank mapping ----
    valid = (ti > 0) & (ti < 15)
    a = ea6[valid].reshape(-1).astype(np.int64)
    b = eb6[valid].reshape(-1).astype(np.int64)
    key = a * NV + b
    n = key.shape[0]
    # pack the slot index into the low 23 bits (n < 2^23, key < 2^36) so a
    # single in-place sort yields both sorted keys and the permutation
    packed = (key << 23) | np.arange(n, dtype=np.int64)
    packed.sort()
    sk = packed >> 23
    order = packed & ((1 << 23) - 1)
    new = np.empty(n, np.bool_)
    new[0] = True
    np.not_equal(sk[1:], sk[:-1], out=new[1:])
    uid = np.cumsum(new) - 1                 # rank among uniques, sorted order
    inv = np.empty(n, np.int64)
    inv[order] = uid
    uk = sk[new]
    ua = (uk // NV).astype(np.int32)
    ub = (uk - (uk // NV) * NV).astype(np.int32)
    cross = occ[ua] != occ[ub]
    mapping = np.where(cross, np.cumsum(cross, dtype=np.int64) - 1,
                       -1).astype(np.int32)
    idx_map = mapping[inv].reshape(-1, 6)
    ia = ua[cross]
    ib = ub[cross]                           # unique crossing edges, lex order
    ne = ia.shape[0]

    # ---- device B: interpolation + uv grid ----
    per = -(-ne // NCORES)                   # ceil
    ek = -(-per // P)
    ek = max(ek, 1)
    epc = P * ek
    if ("interp", ek) not in _prog_cache:
        _prog_cache[("interp", ek)] = _build_interp_program(ek)
    nc_b = _prog_cache[("interp", ek)]

    pos4 = np.concatenate([pos, sdf[:, None]], 1)        # [NV, 4]
    ia_p = np.zeros(epc * NCORES, np.int32)
    ib_p = np.zeros(epc * NCORES, np.int32)
    ia_p[:ne] = ia
    ib_p[:ne] = ib
    recA = pos4[ia_p]
    recB = pos4[ib_p]

    lin = np.linspace(0.0, 1.0 - 1.0 / UVN, UVN, dtype=np.float32)
    lin_bcast = np.ascontiguousarray(np.broadcast_to(lin, (P, UVN)))
    lini_all = np.zeros(NCORES * P, np.float32)
    lini_all[:UVN] = lin
    in_maps = []
    for c in range(NCORES):
        in_maps.append({
            "recA": recA[c * epc:(c + 1) * epc],
            "recB": recB[c * epc:(c + 1) * epc],
            "linj": lin_bcast,
            "lini": lini_all[c * P:(c + 1) * P],
        })
    res_b = _run(nc_b, in_maps, "interp")
    verts = np.concatenate([r["verts"] for r in res_b], 0)[:ne]
    uvrows = np.concatenate([r["uvblk"] for r in res_b], 0)  # [8*128, UVN, 8]
    uvs = uvrows[:UVN].reshape(UVN * UVN * 4, 2)

    # ---- host: faces + uv_idx ----
    ti_v = ti[valid]
    ntri = NUM_TRI_TABLE[ti_v]
    m1 = ntri == 1
    m2 = ntri == 2
    f1 = np.take_along_axis(idx_map[m1], TRIANGLE_TABLE[ti_v[m1]][:, :3],
                            axis=1).reshape(-1, 3)
    f2 = np.take_along_axis(idx_map[m2], TRIANGLE_TABLE[ti_v[m2]][:, :6],
                            axis=1).reshape(-1, 3)
    faces = np.concatenate([f1, f2], 0).astype(np.int32)

    tg = np.arange(NT, dtype=np.int32)[valid]
    fg = np.concatenate([tg[m1] * 2,
                         np.stack([tg[m2] * 2, tg[m2] * 2 + 1],
                                  axis=-1).reshape(-1)], 0)
    t_ = fg // 2
    r_ = fg % 2
    uv_idx = np.stack([t_ * 4, t_ * 4 + r_ + 1, t_ * 4 + r_ + 2],
                      axis=-1).astype(np.int32)

    return verts, faces, uvs, uv_idx


# revision 20
# speedup vs baseline: 1.2315x; 1.2315x over previous
"""DMTet marching-tetrahedra geometry kernel for 8 Trainium2 NeuronCores.

Pipeline (matches reference semantics exactly):
  device A (tet-sharded, 8 cores): occ4 -> tetindex, 6 sorted edge pairs/tet
  host:    valid filter, 36-bit edge-key sort/dedup (rank order), crossing
           mask, cumsum rank mapping, record gathers for interpolation
  device B (edge-sharded, 8 cores): per-edge linear interpolation of the
           crossing point (verts) + uv grid generation
  host:    triangle-table face assembly, uv_idx arithmetic, output assembly
"""

import os
import numpy as np

import concourse.bass as bass
import concourse.tile as tile
from concourse import mybir
from concourse.bass_utils import run_bass_kernel_spmd

NV = 200_000
NT = 800_000
NCORES = 8
P = 128

TRIANGLE_TABLE = np.array([
    [-1, -1, -1, -1, -1, -1], [1, 0, 2, -1, -1, -1], [4, 0, 3, -1, -1, -1],
    [1, 4, 2, 1, 3, 4], [3, 1, 5, -1, -1, -1], [2, 3, 0, 2, 5, 3],
    [1, 4, 0, 1, 5, 4], [4, 2, 5, -1, -1, -1], [4, 5, 2, -1, -1, -1],
    [4, 1, 0, 4, 5, 1], [3, 2, 0, 3, 5, 2], [1, 3, 5, -1, -1, -1],
    [4, 1, 2, 4, 3, 1], [3, 0, 4, -1, -1, -1], [2, 0, 1, -1, -1, -1],
    [-1, -1, -1, -1, -1, -1]], dtype=np.int32)
NUM_TRI_TABLE = np.array([0, 1, 1, 2, 1, 2, 2, 1, 1, 2, 2, 1, 2, 1, 1, 0],
                         dtype=np.int32)
EDGE_PAIRS = [(0, 1), (0, 2), (0, 3), (1, 2), (1, 3), (2, 3)]

# Per-core tet shard: NT/8 = 100000, padded to a multiple of 128.
NTPC = 100_000
TETK = (NTPC + P - 1) // P          # 782 -> padded count 100096
NTPC_PAD = P * TETK

_TRACE = bool(os.environ.get("DMTET_KERNEL_TRACE"))
LAST_EXEC_NS = {}                    # program name -> exec_time_ns (when tracing)

# ---------------------------------------------------------------------------
# walrus in this toolchain accepts at most ONE sync wait per instruction.
# Split multi-wait instructions: excess waits move onto injected
# wait-only InstEventSemaphore instructions placed just before, same engine.
# ---------------------------------------------------------------------------
_MAX_WAITS = 1
_wsplit_uid = [0]


def _legalize_waits(nc):
    for f in nc.m.functions:
        for bb in f.blocks:
            insts = bb.instructions
            out = []
            changed = False
            for inst in insts:
                si = inst.sync_info
                w = list(si.on_wait) if (si is not None and si.on_wait) else []
                if len(w) > _MAX_WAITS:
                    changed = True
                    extra, keep = w[:-_MAX_WAITS], w[-_MAX_WAITS:]
                    for k in range(0, len(extra), _MAX_WAITS):
                        _wsplit_uid[0] += 1
                        out.append(mybir.InstEventSemaphore(
                            name=f"WSPLIT-{_wsplit_uid[0]}",
                            engine=inst.engine,
                            ins=[], outs=[],
                            sync_info=mybir.SyncInfo(
                                on_wait=extra[k:k + _MAX_WAITS], on_update=[]),
                        ))
                    si.on_wait = keep
                out.append(inst)
            if changed:
                bb.instructions = out


def _maybe_install_trace_hook():
    """Register the axon NTFF profile hook if the image's antenv lacks it."""
    if not _TRACE:
        return
    try:
        import antenv.axon_hooks  # noqa: F401
        return
    except ImportError:
        pass
    try:
        import sys
        import types
        import trn_agent_boot.trn_boot as tb
        hook = tb._ntff_profile_via_ctypes('/opt/axon/libaxon_pjrt.so')
        mod = types.ModuleType('antenv.axon_hooks')
        mod.get_axon_ntff_profile_hook = lambda: hook
        mod.set_axon_ntff_profile_hook = lambda h: None
        import antenv
        antenv.axon_hooks = mod
        sys.modules['antenv.axon_hooks'] = mod
    except Exception:
        pass


# ---------------------------------------------------------------------------
# Device program A: per-tet stage.
#   in : tet4 [NTPC_PAD, 4] i32, occ4 [NTPC_PAD, 4] i32 (0/1)
#   out: ea6/eb6 [NTPC_PAD, 6] i32 (sorted edge endpoint pairs),
#        tindex [NTPC_PAD] i32 (4-bit marching-tets code)
# ---------------------------------------------------------------------------
_prog_cache = {}


_BUILDERS_SRC = r'''
def _build_tet_program():
    nc = bass.Bass("TRN2", target_bir_lowering=False, debug=False,
                   num_devices=NCORES, disable_frame_to_traceback=True)
    i32 = mybir.dt.int32
    tet4 = nc.dram_tensor("tet4", [NTPC_PAD, 4], i32, kind="ExternalInput")
    occ4 = nc.dram_tensor("occ4", [NTPC_PAD, 4], mybir.dt.uint8,
                          kind="ExternalInput")
    ea6 = nc.dram_tensor("ea6", [NTPC_PAD, 6], i32, kind="ExternalOutput")
    eb6 = nc.dram_tensor("eb6", [NTPC_PAD, 6], i32, kind="ExternalOutput")
    tindex = nc.dram_tensor("tindex", [NTPC_PAD], i32, kind="ExternalOutput")

    K = TETK
    tet_v = tet4[:].rearrange("(p k) j -> p k j", p=P)
    occ_v = occ4[:].rearrange("(p k) j -> p k j", p=P)
    ea_v = ea6[:].rearrange("(p k) j -> p k j", p=P)
    eb_v = eb6[:].rearrange("(p k) j -> p k j", p=P)
    ti_v = tindex[:].rearrange("(p k) -> p k", p=P)

    KB = 132
    nblk = (K + KB - 1) // KB
    with tile.TileContext(nc) as tc:
        with tc.tile_pool(name="io", bufs=3) as pio, \
             tc.tile_pool(name="tmp", bufs=3) as ptmp:
            for b in range(nblk):
                k0 = b * KB
                kb = min(KB, K - k0)
                t_t = pio.tile([P, KB, 4], i32, tag="t")
                o_t = pio.tile([P, KB, 4], i32, tag="o")
                nc.sync.dma_start(
                    t_t[:, :kb, :].rearrange("p k j -> p (k j)"),
                    tet_v[:, k0:k0 + kb, :].rearrange("p k j -> p (k j)"))
                nc.gpsimd.dma_start(
                    o_t[:, :kb, :].rearrange("p k j -> p (k j)"),
                    occ_v[:, k0:k0 + kb, :].rearrange("p k j -> p (k j)"))

                ti_t = ptmp.tile([P, KB], i32, tag="ti")
                acc = ptmp.tile([P, KB], i32, tag="acc")
                nc.vector.tensor_copy(ti_t[:, :kb], o_t[:, :kb, 0])
                for j, w in ((1, 2), (2, 4), (3, 8)):
                    nc.vector.tensor_scalar(out=acc[:, :kb], in0=o_t[:, :kb, j],
                                            scalar1=w, scalar2=None,
                                            op0=mybir.AluOpType.mult)
                    nc.vector.tensor_tensor(out=ti_t[:, :kb], in0=ti_t[:, :kb],
                                            in1=acc[:, :kb],
                                            op=mybir.AluOpType.add)
                nc.sync.dma_start(ti_v[:, k0:k0 + kb], ti_t[:, :kb])

                ea_t = pio.tile([P, KB, 6], i32, tag="ea")
                eb_t = pio.tile([P, KB, 6], i32, tag="eb")
                for e, (u, v) in enumerate(EDGE_PAIRS):
                    nc.vector.tensor_tensor(out=ea_t[:, :kb, e],
                                            in0=t_t[:, :kb, u],
                                            in1=t_t[:, :kb, v],
                                            op=mybir.AluOpType.min)
                    nc.vector.tensor_tensor(out=eb_t[:, :kb, e],
                                            in0=t_t[:, :kb, u],
                                            in1=t_t[:, :kb, v],
                                            op=mybir.AluOpType.max)
                nc.sync.dma_start(
                    ea_v[:, k0:k0 + kb, :].rearrange("p k j -> p (k j)"),
                    ea_t[:, :kb, :].rearrange("p k j -> p (k j)"))
                nc.sync.dma_start(
                    eb_v[:, k0:k0 + kb, :].rearrange("p k j -> p (k j)"),
                    eb_t[:, :kb, :].rearrange("p k j -> p (k j)"))
    return nc


def _build_interp_program(ek):
    nc = bass.Bass("TRN2", target_bir_lowering=False, debug=False,
                   num_devices=NCORES, disable_frame_to_traceback=True)
    f32 = mybir.dt.float32
    epc = P * ek
    # SoA planes: row c of recA/recB is component c (x, y, z, sdf)
    recA = nc.dram_tensor("recA", [4, epc], f32, kind="ExternalInput")
    recB = nc.dram_tensor("recB", [4, epc], f32, kind="ExternalInput")
    linj = nc.dram_tensor("linj", [P, UVN], f32, kind="ExternalInput")
    lini = nc.dram_tensor("lini", [P], f32, kind="ExternalInput")
    verts = nc.dram_tensor("verts", [3, epc], f32, kind="ExternalOutput")
    uvblk = nc.dram_tensor("uvblk", [P, UVN, 8], f32, kind="ExternalOutput")

    EKB = 512
    nblk = (ek + EKB - 1) // EKB
    with tile.TileContext(nc) as tc:
        with tc.tile_pool(name="io", bufs=3) as pio, \
             tc.tile_pool(name="tmp", bufs=2) as ptmp:
            # ---- uv grid block ----
            lj = pio.tile([P, UVN], f32, tag="lj")
            nc.sync.dma_start(lj[:], linj[:, :])
            li = pio.tile([P, 1], f32, tag="li")
            nc.sync.dma_start(li[:], lini[:, None])
            uv = pio.tile([P, UVN, 8], f32, tag="uv")
            ljp = ptmp.tile([P, UVN], f32, tag="ljp")
            nc.vector.tensor_scalar(out=ljp[:], in0=lj[:],
                                    scalar1=float(UVPAD_CONST), scalar2=None,
                                    op0=mybir.AluOpType.add)
            lib = ptmp.tile([P, UVN], f32, tag="lib")
            nc.vector.tensor_scalar(out=lib[:], in0=li[:].to_broadcast([P, UVN]),
                                    scalar1=0.0, scalar2=None,
                                    op0=mybir.AluOpType.add)
            libp = ptmp.tile([P, UVN], f32, tag="libp")
            nc.vector.tensor_scalar(out=libp[:], in0=lib[:],
                                    scalar1=float(UVPAD_CONST), scalar2=None,
                                    op0=mybir.AluOpType.add)
            for c, srct in ((0, lj), (1, lib), (2, ljp), (3, lib),
                            (4, ljp), (5, libp), (6, lj), (7, libp)):
                nc.vector.tensor_copy(uv[:, :, c], srct[:])
            nc.sync.dma_start(uvblk[:, :, :].rearrange("p k j -> p (k j)"),
                              uv[:].rearrange("p k j -> p (k j)"))

            # ---- interpolation blocks (all DVE accesses contiguous) ----
            recA_v = recA[:].rearrange("c (p k) -> p c k", p=P)
            recB_v = recB[:].rearrange("c (p k) -> p c k", p=P)
            verts_v = verts[:].rearrange("c (p k) -> p c k", p=P)
            for b in range(nblk):
                k0 = b * EKB
                kb = min(EKB, ek - k0)
                a_t = pio.tile([P, 4, EKB], f32, tag="a")
                b_t = pio.tile([P, 4, EKB], f32, tag="b")
                nc.sync.dma_start(a_t[:, :, :kb], recA_v[:, :, k0:k0 + kb])
                nc.sync.dma_start(b_t[:, :, :kb], recB_v[:, :, k0:k0 + kb])
                d_t = ptmp.tile([P, EKB], f32, tag="d")
                nc.vector.tensor_tensor(out=d_t[:, :kb], in0=a_t[:, 3, :kb],
                                        in1=b_t[:, 3, :kb],
                                        op=mybir.AluOpType.subtract)
                r_t = ptmp.tile([P, EKB], f32, tag="r")
                nc.vector.reciprocal(r_t[:, :kb], d_t[:, :kb])
                nt_t = ptmp.tile([P, EKB], f32, tag="nt")
                nc.vector.tensor_tensor(out=nt_t[:, :kb], in0=d_t[:, :kb],
                                        in1=r_t[:, :kb],
                                        op=mybir.AluOpType.mult)
                nc.vector.tensor_scalar(out=nt_t[:, :kb], in0=nt_t[:, :kb],
                                        scalar1=-1.0, scalar2=2.0,
                                        op0=mybir.AluOpType.mult,
                                        op1=mybir.AluOpType.add)
                nc.vector.tensor_tensor(out=r_t[:, :kb], in0=r_t[:, :kb],
                                        in1=nt_t[:, :kb],
                                        op=mybir.AluOpType.mult)
                wa_t = ptmp.tile([P, EKB], f32, tag="wa")
                nc.vector.tensor_scalar(out=wa_t[:, :kb], in0=b_t[:, 3, :kb],
                                        scalar1=-1.0, scalar2=None,
                                        op0=mybir.AluOpType.mult)
                nc.vector.tensor_tensor(out=wa_t[:, :kb], in0=wa_t[:, :kb],
                                        in1=r_t[:, :kb],
                                        op=mybir.AluOpType.mult)
                wb_t = ptmp.tile([P, EKB], f32, tag="wb")
                nc.vector.tensor_tensor(out=wb_t[:, :kb], in0=a_t[:, 3, :kb],
                                        in1=r_t[:, :kb],
                                        op=mybir.AluOpType.mult)
                v_t = pio.tile([P, 3, EKB], f32, tag="v")
                m_t = ptmp.tile([P, EKB], f32, tag="m")
                for c in range(3):
                    nc.vector.tensor_tensor(out=m_t[:, :kb],
                                            in0=a_t[:, c, :kb],
                                            in1=wa_t[:, :kb],
                                            op=mybir.AluOpType.mult)
                    nc.vector.tensor_tensor(out=v_t[:, c, :kb],
                                            in0=b_t[:, c, :kb],
                                            in1=wb_t[:, :kb],
                                            op=mybir.AluOpType.mult)
                    nc.vector.tensor_tensor(out=v_t[:, c, :kb],
                                            in0=v_t[:, c, :kb],
                                            in1=m_t[:, :kb],
                                            op=mybir.AluOpType.add)
                nc.sync.dma_start(verts_v[:, :, k0:k0 + kb], v_t[:, :, :kb])
    return nc
'''

# exec with a fixed pseudo-filename so the BIR debug table (and hence the
# NEFF cache key) is independent of the directory kernel.py runs from
UVN = 895          # ceil(sqrt((2*NT+1)//2))
UVPAD_CONST = np.float32(0.9 / UVN)

exec(compile(_BUILDERS_SRC, "dmtet_builders.py", "exec"), globals())

# Normalize instruction/allocation debug info the same way: caller frames
# otherwise leak harness file paths into the serialized BIR.
_orig_get_debug_info = bass.Bass.get_debug_info


def _stable_get_debug_info(self):
    d = _orig_get_debug_info(self)
    return mybir.OpDebugInfo(
        op_name=d.op_name, tensorizer_id=d.tensorizer_id,
        filename=d.filename if d.filename == "dmtet_builders.py"
        else "kernel.py",
        lineno=d.lineno, bass_funcname=d.bass_funcname,
        kernel_name=d.kernel_name, ant_traceback="",
        ant_layer=d.ant_layer, ant_annotation=d.ant_annotation)


bass.Bass.get_debug_info = _stable_get_debug_info


_exec_cache = {}


def _cached_spmd_run(nc, in_maps):
    """Like bass2jax.run_bass_via_pjrt but with the jitted executable cached
    across kernel() calls (a fresh jit per call costs seconds in NEFF
    load/dispatch)."""
    import jax
    from jax.sharding import Mesh, PartitionSpec
    try:
        from jax.experimental.shard_map import shard_map
    except ImportError:
        from jax.shard_map import shard_map
    from concourse import bass2jax

    key = id(nc)
    if key not in _exec_cache:
        bass2jax.install_neuronx_cc_hook()
        partition_name = (nc.partition_id_tensor.name
                          if nc.partition_id_tensor else None)
        in_names, out_names, out_avals = [], [], []
        for alloc in nc.m.functions[0].allocations:
            if not isinstance(alloc, mybir.MemoryLocationSet):
                continue
            name = alloc.memorylocations[0].name
            if alloc.kind == "ExternalInput":
                if name != partition_name:
                    in_names.append(name)
            elif alloc.kind == "ExternalOutput":
                out_names.append(name)
                out_avals.append(jax.core.ShapedArray(
                    tuple(alloc.tensor_shape), mybir.dt.np(alloc.dtype)))
        n_params = len(in_names)
        all_names = list(in_names) + list(out_names)
        if partition_name is not None:
            all_names.append(partition_name)

        def _body(*args):
            operands = list(args)
            if partition_name is not None:
                operands.append(bass2jax.partition_id_tensor())
            outs = bass2jax._bass_exec_p.bind(
                *operands,
                out_avals=tuple(out_avals),
                in_names=tuple(all_names),
                out_names=tuple(out_names),
                lowering_input_output_aliases=(),
                sim_require_finite=True,
                sim_require_nnan=True,
                nc=nc,
            )
            return tuple(outs)

        devices = jax.devices()[:NCORES]
        mesh = Mesh(np.asarray(devices), ("core",))
        n_outs = len(out_names)
        sharded = jax.jit(
            shard_map(_body, mesh=mesh,
                      in_specs=(PartitionSpec("core"),) * (n_params + n_outs),
                      out_specs=(PartitionSpec("core"),) * n_outs,
                      check_rep=False),
            donate_argnums=tuple(range(n_params, n_params + n_outs)),
            keep_unused=True)
        _exec_cache[key] = (sharded, in_names, out_names, out_avals, n_params)

    sharded, in_names, out_names, out_avals, n_params = _exec_cache[key]
    concat_in = [np.concatenate([np.asarray(m[name]) for m in in_maps], axis=0)
                 for name in in_names]
    concat_zeros = [np.zeros((NCORES * a.shape[0], *a.shape[1:]), a.dtype)
                    for a in out_avals]
    out_arrs = sharded(*concat_in, *concat_zeros)
    return [{name: np.asarray(out_arrs[i]).reshape(
                NCORES, *out_avals[i].shape)[c]
             for i, name in enumerate(out_names)}
            for c in range(NCORES)]


def _run(nc, in_maps, name):
    if not getattr(nc, "_waits_legalized", False):
        _legalize_waits(nc)
        nc._waits_legalized = True
    if _TRACE:
        _maybe_install_trace_hook()
        res = run_bass_kernel_spmd(nc, in_maps, core_ids=list(range(NCORES)),
                                   trace=True)
        if res.exec_time_ns is not None:
            LAST_EXEC_NS[name] = res.exec_time_ns
        return res.results
    return _cached_spmd_run(nc, in_maps)


# ---------------------------------------------------------------------------
# Host orchestration
# ---------------------------------------------------------------------------

def kernel(pos_nx3, sdf_n, tet_fx4):
    pos = np.ascontiguousarray(np.asarray(pos_nx3, dtype=np.float32))
    sdf = np.ascontiguousarray(np.asarray(sdf_n, dtype=np.float32))
    tet = np.ascontiguousarray(np.asarray(tet_fx4, dtype=np.int32))

    occ = sdf > 0
    occ4 = occ[tet].astype(np.uint8)                     # [NT, 4]

    # ---- device A: tetindex + sorted edge pairs ----
    if "tet" not in _prog_cache:
        _prog_cache["tet"] = _build_tet_program()
    nc_a = _prog_cache["tet"]
    pad = NTPC_PAD * NCORES - NT
    tet_p = np.concatenate([tet, np.zeros((pad, 4), np.int32)], 0)
    occ4_p = np.concatenate([occ4, np.zeros((pad, 4), np.uint8)], 0)
    in_maps = [{"tet4": tet_p[c * NTPC_PAD:(c + 1) * NTPC_PAD],
                "occ4": occ4_p[c * NTPC_PAD:(c + 1) * NTPC_PAD]}
               for c in range(NCORES)]
    res_a = _run(nc_a, in_maps, "tet")
    ea6 = np.concatenate([r["ea6"] for r in res_a], 0)[:NT]
    eb6 = np.concatenate([r["eb6"] for r in res_a], 0)[:NT]
    ti = np.concatenate([r["tindex"] for r in res_a], 0)[:NT]

    # ---- host: valid filter + sort/dedup + rank mapping ----
    valid = (ti > 0) & (ti < 15)
    a = ea6[valid].reshape(-1).astype(np.int64)
    b = eb6[valid].reshape(-1).astype(np.int64)
    key = a * NV + b
    n = key.shape[0]
    # pack the slot index into the low 23 bits (n < 2^23, key < 2^36) so a
    # single in-place sort yields both sorted keys and the permutation
    packed = (key << 23) | np.arange(n, dtype=np.int64)
    packed.sort()
    sk = packed >> 23
    order = packed & ((1 << 23) - 1)
    new = np.empty(n, np.bool_)
    new[0] = True
    np.not_equal(sk[1:], sk[:-1], out=new[1:])
    uid = np.cumsum(new) - 1                 # rank among uniques, sorted order
    inv = np.empty(n, np.int64)
    inv[order] = uid
    uk = sk[new]
    ua = (uk // NV).astype(np.int32)
    ub = (uk - (uk // NV) * NV).astype(np.int32)
    cross = occ[ua] != occ[ub]
    mapping = np.where(cross, np.cumsum(cross, dtype=np.int64) - 1,
                       -1).astype(np.int32)
    idx_map = mapping[inv].reshape(-1, 6)
    ia = ua[cross]
    ib = ub[cross]                           # unique crossing edges, lex order
    ne = ia.shape[0]

    # ---- device B: interpolation + uv grid ----
    per = -(-ne // NCORES)                   # ceil
    ek = -(-per // P)
    ek = max(ek, 1)
    epc = P * ek
    if ("interp", ek) not in _prog_cache:
        _prog_cache[("interp", ek)] = _build_interp_program(ek)
    nc_b = _prog_cache[("interp", ek)]

    posT = np.empty((4, NV), np.float32)                 # SoA planes
    posT[:3] = pos.T
    posT[3] = sdf
    ia_p = np.zeros(epc * NCORES, np.int32)
    ib_p = np.zeros(epc * NCORES, np.int32)
    ia_p[:ne] = ia
    ib_p[:ne] = ib
    recA = posT[:, ia_p]                                 # [4, 8*epc]
    recB = posT[:, ib_p]

    lin = np.linspace(0.0, 1.0 - 1.0 / UVN, UVN, dtype=np.float32)
    lin_bcast = np.ascontiguousarray(np.broadcast_to(lin, (P, UVN)))
    lini_all = np.zeros(NCORES * P, np.float32)
    lini_all[:UVN] = lin
    in_maps = []
    for c in range(NCORES):
        in_maps.append({
            "recA": np.ascontiguousarray(recA[:, c * epc:(c + 1) * epc]),
            "recB": np.ascontiguousarray(recB[:, c * epc:(c + 1) * epc]),
            "linj": lin_bcast,
            "lini": lini_all[c * P:(c + 1) * P],
        })
    res_b = _run(nc_b, in_maps, "interp")
    verts = np.ascontiguousarray(
        np.concatenate([r["verts"] for r in res_b], 1)[:, :ne].T)
    uvrows = np.concatenate([r["uvblk"] for r in res_b], 0)  # [8*128, UVN, 8]
    uvs = uvrows[:UVN].reshape(UVN * UVN * 4, 2)

    # ---- host: faces + uv_idx ----
    ti_v = ti[valid]
    ntri = NUM_TRI_TABLE[ti_v]
    m1 = ntri == 1
    m2 = ntri == 2
    f1 = np.take_along_axis(idx_map[m1], TRIANGLE_TABLE[ti_v[m1]][:, :3],
                            axis=1).reshape(-1, 3)
    f2 = np.take_along_axis(idx_map[m2], TRIANGLE_TABLE[ti_v[m2]][:, :6],
                            axis=1).reshape(-1, 3)
    faces = np.concatenate([f1, f2], 0).astype(np.int32)

    tg = np.arange(NT, dtype=np.int32)[valid]
    fg = np.concatenate([tg[m1] * 2,
                         np.stack([tg[m2] * 2, tg[m2] * 2 + 1],
                                  axis=-1).reshape(-1)], 0)
    t_ = fg // 2
    r_ = fg % 2
    uv_idx = np.stack([t_ * 4, t_ * 4 + r_ + 1, t_ * 4 + r_ + 2],
                      axis=-1).astype(np.int32)

    return verts, faces, uvs, uv_idx


# revision 21
# speedup vs baseline: 1.2953x; 1.0518x over previous
"""DMTet marching-tetrahedra geometry kernel for 8 Trainium2 NeuronCores.

Pipeline (matches reference semantics exactly):
  device A (tet-sharded, 8 cores): occ4 -> tetindex, 6 sorted edge pairs/tet
  host:    valid filter, 36-bit edge-key sort/dedup (rank order), crossing
           mask, cumsum rank mapping, record gathers for interpolation
  device B (edge-sharded, 8 cores): per-edge linear interpolation of the
           crossing point (verts) + uv grid generation
  host:    triangle-table face assembly, uv_idx arithmetic, output assembly
"""

import os
import numpy as np

import concourse.bass as bass
import concourse.tile as tile
from concourse import mybir
from concourse.bass_utils import run_bass_kernel_spmd

NV = 200_000
NT = 800_000
NCORES = 8
P = 128

TRIANGLE_TABLE = np.array([
    [-1, -1, -1, -1, -1, -1], [1, 0, 2, -1, -1, -1], [4, 0, 3, -1, -1, -1],
    [1, 4, 2, 1, 3, 4], [3, 1, 5, -1, -1, -1], [2, 3, 0, 2, 5, 3],
    [1, 4, 0, 1, 5, 4], [4, 2, 5, -1, -1, -1], [4, 5, 2, -1, -1, -1],
    [4, 1, 0, 4, 5, 1], [3, 2, 0, 3, 5, 2], [1, 3, 5, -1, -1, -1],
    [4, 1, 2, 4, 3, 1], [3, 0, 4, -1, -1, -1], [2, 0, 1, -1, -1, -1],
    [-1, -1, -1, -1, -1, -1]], dtype=np.int32)
NUM_TRI_TABLE = np.array([0, 1, 1, 2, 1, 2, 2, 1, 1, 2, 2, 1, 2, 1, 1, 0],
                         dtype=np.int32)
EDGE_PAIRS = [(0, 1), (0, 2), (0, 3), (1, 2), (1, 3), (2, 3)]

# Per-core tet shard: NT/8 = 100000, padded to a multiple of 128.
NTPC = 100_000
TETK = (NTPC + P - 1) // P          # 782 -> padded count 100096
NTPC_PAD = P * TETK

_TRACE = bool(os.environ.get("DMTET_KERNEL_TRACE"))
LAST_EXEC_NS = {}                    # program name -> exec_time_ns (when tracing)

# ---------------------------------------------------------------------------
# walrus in this toolchain accepts at most ONE sync wait per instruction.
# Split multi-wait instructions: excess waits move onto injected
# wait-only InstEventSemaphore instructions placed just before, same engine.
# ---------------------------------------------------------------------------
_MAX_WAITS = 1
_wsplit_uid = [0]


def _legalize_waits(nc):
    for f in nc.m.functions:
        for bb in f.blocks:
            insts = bb.instructions
            out = []
            changed = False
            for inst in insts:
                si = inst.sync_info
                w = list(si.on_wait) if (si is not None and si.on_wait) else []
                if len(w) > _MAX_WAITS:
                    changed = True
                    extra, keep = w[:-_MAX_WAITS], w[-_MAX_WAITS:]
                    for k in range(0, len(extra), _MAX_WAITS):
                        _wsplit_uid[0] += 1
                        out.append(mybir.InstEventSemaphore(
                            name=f"WSPLIT-{_wsplit_uid[0]}",
                            engine=inst.engine,
                            ins=[], outs=[],
                            sync_info=mybir.SyncInfo(
                                on_wait=extra[k:k + _MAX_WAITS], on_update=[]),
                        ))
                    si.on_wait = keep
                out.append(inst)
            if changed:
                bb.instructions = out


def _maybe_install_trace_hook():
    """Register the axon NTFF profile hook if the image's antenv lacks it."""
    if not _TRACE:
        return
    try:
        import antenv.axon_hooks  # noqa: F401
        return
    except ImportError:
        pass
    try:
        import sys
        import types
        import trn_agent_boot.trn_boot as tb
        hook = tb._ntff_profile_via_ctypes('/opt/axon/libaxon_pjrt.so')
        mod = types.ModuleType('antenv.axon_hooks')
        mod.get_axon_ntff_profile_hook = lambda: hook
        mod.set_axon_ntff_profile_hook = lambda h: None
        import antenv
        antenv.axon_hooks = mod
        sys.modules['antenv.axon_hooks'] = mod
    except Exception:
        pass


# ---------------------------------------------------------------------------
# Device program A: per-tet stage.
#   in : tet4 [NTPC_PAD, 4] i32, occ4 [NTPC_PAD, 4] i32 (0/1)
#   out: ea6/eb6 [NTPC_PAD, 6] i32 (sorted edge endpoint pairs),
#        tindex [NTPC_PAD] i32 (4-bit marching-tets code)
# ---------------------------------------------------------------------------
_prog_cache = {}


_BUILDERS_SRC = r'''
def _build_tet_program():
    nc = bass.Bass("TRN2", target_bir_lowering=False, debug=False,
                   num_devices=NCORES, disable_frame_to_traceback=True)
    i32 = mybir.dt.int32
    tet4 = nc.dram_tensor("tet4", [NTPC_PAD, 4], i32, kind="ExternalInput")
    occ4 = nc.dram_tensor("occ4", [NTPC_PAD, 4], mybir.dt.uint8,
                          kind="ExternalInput")
    ea6 = nc.dram_tensor("ea6", [NTPC_PAD, 6], i32, kind="ExternalOutput")
    eb6 = nc.dram_tensor("eb6", [NTPC_PAD, 6], i32, kind="ExternalOutput")
    tindex = nc.dram_tensor("tindex", [NTPC_PAD], i32, kind="ExternalOutput")

    K = TETK
    tet_v = tet4[:].rearrange("(p k) j -> p k j", p=P)
    occ_v = occ4[:].rearrange("(p k) j -> p k j", p=P)
    ea_v = ea6[:].rearrange("(p k) j -> p k j", p=P)
    eb_v = eb6[:].rearrange("(p k) j -> p k j", p=P)
    ti_v = tindex[:].rearrange("(p k) -> p k", p=P)

    KB = 132
    nblk = (K + KB - 1) // KB
    with tile.TileContext(nc) as tc:
        with tc.tile_pool(name="io", bufs=3) as pio, \
             tc.tile_pool(name="tmp", bufs=3) as ptmp:
            for b in range(nblk):
                k0 = b * KB
                kb = min(KB, K - k0)
                t_t = pio.tile([P, KB, 4], i32, tag="t")
                o_t = pio.tile([P, KB, 4], i32, tag="o")
                nc.sync.dma_start(
                    t_t[:, :kb, :].rearrange("p k j -> p (k j)"),
                    tet_v[:, k0:k0 + kb, :].rearrange("p k j -> p (k j)"))
                nc.gpsimd.dma_start(
                    o_t[:, :kb, :].rearrange("p k j -> p (k j)"),
                    occ_v[:, k0:k0 + kb, :].rearrange("p k j -> p (k j)"))

                ti_t = ptmp.tile([P, KB], i32, tag="ti")
                acc = ptmp.tile([P, KB], i32, tag="acc")
                nc.vector.tensor_copy(ti_t[:, :kb], o_t[:, :kb, 0])
                for j, w in ((1, 2), (2, 4), (3, 8)):
                    nc.vector.tensor_scalar(out=acc[:, :kb], in0=o_t[:, :kb, j],
                                            scalar1=w, scalar2=None,
                                            op0=mybir.AluOpType.mult)
                    nc.vector.tensor_tensor(out=ti_t[:, :kb], in0=ti_t[:, :kb],
                                            in1=acc[:, :kb],
                                            op=mybir.AluOpType.add)
                nc.gpsimd.dma_start(ti_v[:, k0:k0 + kb], ti_t[:, :kb])

                ea_t = pio.tile([P, KB, 6], i32, tag="ea")
                eb_t = pio.tile([P, KB, 6], i32, tag="eb")
                for e, (u, v) in enumerate(EDGE_PAIRS):
                    nc.vector.tensor_tensor(out=ea_t[:, :kb, e],
                                            in0=t_t[:, :kb, u],
                                            in1=t_t[:, :kb, v],
                                            op=mybir.AluOpType.min)
                    nc.vector.tensor_tensor(out=eb_t[:, :kb, e],
                                            in0=t_t[:, :kb, u],
                                            in1=t_t[:, :kb, v],
                                            op=mybir.AluOpType.max)
                nc.gpsimd.dma_start(
                    ea_v[:, k0:k0 + kb, :].rearrange("p k j -> p (k j)"),
                    ea_t[:, :kb, :].rearrange("p k j -> p (k j)"))
                nc.gpsimd.dma_start(
                    eb_v[:, k0:k0 + kb, :].rearrange("p k j -> p (k j)"),
                    eb_t[:, :kb, :].rearrange("p k j -> p (k j)"))
    return nc


def _build_interp_program(ek):
    nc = bass.Bass("TRN2", target_bir_lowering=False, debug=False,
                   num_devices=NCORES, disable_frame_to_traceback=True)
    f32 = mybir.dt.float32
    epc = P * ek
    # SoA planes: row c of recA/recB is component c (x, y, z, sdf)
    recA = nc.dram_tensor("recA", [4, epc], f32, kind="ExternalInput")
    recB = nc.dram_tensor("recB", [4, epc], f32, kind="ExternalInput")
    linj = nc.dram_tensor("linj", [P, UVN], f32, kind="ExternalInput")
    lini = nc.dram_tensor("lini", [P], f32, kind="ExternalInput")
    verts = nc.dram_tensor("verts", [3, epc], f32, kind="ExternalOutput")
    uvblk = nc.dram_tensor("uvblk", [P, UVN, 8], f32, kind="ExternalOutput")

    EKB = 512
    nblk = (ek + EKB - 1) // EKB
    with tile.TileContext(nc) as tc:
        with tc.tile_pool(name="io", bufs=4) as pio, \
             tc.tile_pool(name="uvp", bufs=1) as puv, \
             tc.tile_pool(name="tmp", bufs=2) as ptmp:
            # ---- uv grid block ----
            lj = puv.tile([P, UVN], f32, tag="lj")
            nc.sync.dma_start(lj[:], linj[:, :])
            li = puv.tile([P, 1], f32, tag="li")
            nc.sync.dma_start(li[:], lini[:, None])
            uv = puv.tile([P, UVN, 8], f32, tag="uv")
            ljp = puv.tile([P, UVN], f32, tag="ljp")
            nc.vector.tensor_scalar(out=ljp[:], in0=lj[:],
                                    scalar1=float(UVPAD_CONST), scalar2=None,
                                    op0=mybir.AluOpType.add)
            lib = puv.tile([P, UVN], f32, tag="lib")
            nc.vector.tensor_scalar(out=lib[:], in0=li[:].to_broadcast([P, UVN]),
                                    scalar1=0.0, scalar2=None,
                                    op0=mybir.AluOpType.add)
            libp = puv.tile([P, UVN], f32, tag="libp")
            nc.vector.tensor_scalar(out=libp[:], in0=lib[:],
                                    scalar1=float(UVPAD_CONST), scalar2=None,
                                    op0=mybir.AluOpType.add)
            for c, srct in ((0, lj), (1, lib), (2, ljp), (3, lib),
                            (4, ljp), (5, libp), (6, lj), (7, libp)):
                nc.vector.tensor_copy(uv[:, :, c], srct[:])
            nc.gpsimd.dma_start(uvblk[:, :, :].rearrange("p k j -> p (k j)"),
                              uv[:].rearrange("p k j -> p (k j)"))

            # ---- interpolation blocks (all DVE accesses contiguous) ----
            recA_v = recA[:].rearrange("c (p k) -> p c k", p=P)
            recB_v = recB[:].rearrange("c (p k) -> p c k", p=P)
            verts_v = verts[:].rearrange("c (p k) -> p c k", p=P)
            for b in range(nblk):
                k0 = b * EKB
                kb = min(EKB, ek - k0)
                a_t = pio.tile([P, 4, EKB], f32, tag="a")
                b_t = pio.tile([P, 4, EKB], f32, tag="b")
                nc.sync.dma_start(a_t[:, :, :kb], recA_v[:, :, k0:k0 + kb])
                nc.sync.dma_start(b_t[:, :, :kb], recB_v[:, :, k0:k0 + kb])
                d_t = ptmp.tile([P, EKB], f32, tag="d")
                nc.vector.tensor_tensor(out=d_t[:, :kb], in0=a_t[:, 3, :kb],
                                        in1=b_t[:, 3, :kb],
                                        op=mybir.AluOpType.subtract)
                r_t = ptmp.tile([P, EKB], f32, tag="r")
                nc.vector.reciprocal(r_t[:, :kb], d_t[:, :kb])
                nt_t = ptmp.tile([P, EKB], f32, tag="nt")
                nc.vector.tensor_tensor(out=nt_t[:, :kb], in0=d_t[:, :kb],
                                        in1=r_t[:, :kb],
                                        op=mybir.AluOpType.mult)
                nc.vector.tensor_scalar(out=nt_t[:, :kb], in0=nt_t[:, :kb],
                                        scalar1=-1.0, scalar2=2.0,
                                        op0=mybir.AluOpType.mult,
                                        op1=mybir.AluOpType.add)
                nc.vector.tensor_tensor(out=r_t[:, :kb], in0=r_t[:, :kb],
                                        in1=nt_t[:, :kb],
                                        op=mybir.AluOpType.mult)
                wa_t = ptmp.tile([P, EKB], f32, tag="wa")
                nc.vector.tensor_scalar(out=wa_t[:, :kb], in0=b_t[:, 3, :kb],
                                        scalar1=-1.0, scalar2=None,
                                        op0=mybir.AluOpType.mult)
                nc.vector.tensor_tensor(out=wa_t[:, :kb], in0=wa_t[:, :kb],
                                        in1=r_t[:, :kb],
                                        op=mybir.AluOpType.mult)
                wb_t = ptmp.tile([P, EKB], f32, tag="wb")
                nc.vector.tensor_tensor(out=wb_t[:, :kb], in0=a_t[:, 3, :kb],
                                        in1=r_t[:, :kb],
                                        op=mybir.AluOpType.mult)
                v_t = pio.tile([P, 3, EKB], f32, tag="v")
                m_t = ptmp.tile([P, EKB], f32, tag="m")
                for c in range(3):
                    nc.vector.tensor_tensor(out=m_t[:, :kb],
                                            in0=a_t[:, c, :kb],
                                            in1=wa_t[:, :kb],
                                            op=mybir.AluOpType.mult)
                    nc.vector.tensor_tensor(out=v_t[:, c, :kb],
                                            in0=b_t[:, c, :kb],
                                            in1=wb_t[:, :kb],
                                            op=mybir.AluOpType.mult)
                    nc.vector.tensor_tensor(out=v_t[:, c, :kb],
                                            in0=v_t[:, c, :kb],
                                            in1=m_t[:, :kb],
                                            op=mybir.AluOpType.add)
                nc.gpsimd.dma_start(verts_v[:, :, k0:k0 + kb], v_t[:, :, :kb])
    return nc
'''

# exec with a fixed pseudo-filename so the BIR debug table (and hence the
# NEFF cache key) is independent of the directory kernel.py runs from
UVN = 895          # ceil(sqrt((2*NT+1)//2))
UVPAD_CONST = np.float32(0.9 / UVN)

exec(compile(_BUILDERS_SRC, "dmtet_builders.py", "exec"), globals())

# Normalize instruction/allocation debug info the same way: caller frames
# otherwise leak harness file paths into the serialized BIR.
_orig_get_debug_info = bass.Bass.get_debug_info


def _stable_get_debug_info(self):
    d = _orig_get_debug_info(self)
    return mybir.OpDebugInfo(
        op_name=d.op_name, tensorizer_id=d.tensorizer_id,
        filename=d.filename if d.filename == "dmtet_builders.py"
        else "kernel.py",
        lineno=d.lineno, bass_funcname=d.bass_funcname,
        kernel_name=d.kernel_name, ant_traceback="",
        ant_layer=d.ant_layer, ant_annotation=d.ant_annotation)


bass.Bass.get_debug_info = _stable_get_debug_info


_exec_cache = {}


def _cached_spmd_run(nc, in_maps):
    """Like bass2jax.run_bass_via_pjrt but with the jitted executable cached
    across kernel() calls (a fresh jit per call costs seconds in NEFF
    load/dispatch)."""
    import jax
    from jax.sharding import Mesh, PartitionSpec
    try:
        from jax.experimental.shard_map import shard_map
    except ImportError:
        from jax.shard_map import shard_map
    from concourse import bass2jax

    key = id(nc)
    if key not in _exec_cache:
        bass2jax.install_neuronx_cc_hook()
        partition_name = (nc.partition_id_tensor.name
                          if nc.partition_id_tensor else None)
        in_names, out_names, out_avals = [], [], []
        for alloc in nc.m.functions[0].allocations:
            if not isinstance(alloc, mybir.MemoryLocationSet):
                continue
            name = alloc.memorylocations[0].name
            if alloc.kind == "ExternalInput":
                if name != partition_name:
                    in_names.append(name)
            elif alloc.kind == "ExternalOutput":
                out_names.append(name)
                out_avals.append(jax.core.ShapedArray(
                    tuple(alloc.tensor_shape), mybir.dt.np(alloc.dtype)))
        n_params = len(in_names)
        all_names = list(in_names) + list(out_names)
        if partition_name is not None:
            all_names.append(partition_name)

        def _body(*args):
            operands = list(args)
            if partition_name is not None:
                operands.append(bass2jax.partition_id_tensor())
            outs = bass2jax._bass_exec_p.bind(
                *operands,
                out_avals=tuple(out_avals),
                in_names=tuple(all_names),
                out_names=tuple(out_names),
                lowering_input_output_aliases=(),
                sim_require_finite=True,
                sim_require_nnan=True,
                nc=nc,
            )
            return tuple(outs)

        devices = jax.devices()[:NCORES]
        mesh = Mesh(np.asarray(devices), ("core",))
        n_outs = len(out_names)
        sharded = jax.jit(
            shard_map(_body, mesh=mesh,
                      in_specs=(PartitionSpec("core"),) * (n_params + n_outs),
                      out_specs=(PartitionSpec("core"),) * n_outs,
                      check_rep=False),
            donate_argnums=tuple(range(n_params, n_params + n_outs)),
            keep_unused=True)
        _exec_cache[key] = (sharded, in_names, out_names, out_avals, n_params)

    sharded, in_names, out_names, out_avals, n_params = _exec_cache[key]
    concat_in = [np.concatenate([np.asarray(m[name]) for m in in_maps], axis=0)
                 for name in in_names]
    concat_zeros = [np.zeros((NCORES * a.shape[0], *a.shape[1:]), a.dtype)
                    for a in out_avals]
    out_arrs = sharded(*concat_in, *concat_zeros)
    return [{name: np.asarray(out_arrs[i]).reshape(
                NCORES, *out_avals[i].shape)[c]
             for i, name in enumerate(out_names)}
            for c in range(NCORES)]


def _run(nc, in_maps, name):
    if not getattr(nc, "_waits_legalized", False):
        _legalize_waits(nc)
        nc._waits_legalized = True
    if _TRACE:
        _maybe_install_trace_hook()
        res = run_bass_kernel_spmd(nc, in_maps, core_ids=list(range(NCORES)),
                                   trace=True)
        if res.exec_time_ns is not None:
            LAST_EXEC_NS[name] = res.exec_time_ns
        return res.results
    return _cached_spmd_run(nc, in_maps)


# ---------------------------------------------------------------------------
# Host orchestration
# ---------------------------------------------------------------------------

def kernel(pos_nx3, sdf_n, tet_fx4):
    pos = np.ascontiguousarray(np.asarray(pos_nx3, dtype=np.float32))
    sdf = np.ascontiguousarray(np.asarray(sdf_n, dtype=np.float32))
    tet = np.ascontiguousarray(np.asarray(tet_fx4, dtype=np.int32))

    occ = sdf > 0
    occ4 = occ[tet].astype(np.uint8)                     # [NT, 4]

    # ---- device A: tetindex + sorted edge pairs ----
    if "tet" not in _prog_cache:
        _prog_cache["tet"] = _build_tet_program()
    nc_a = _prog_cache["tet"]
    pad = NTPC_PAD * NCORES - NT
    tet_p = np.concatenate([tet, np.zeros((pad, 4), np.int32)], 0)
    occ4_p = np.concatenate([occ4, np.zeros((pad, 4), np.uint8)], 0)
    in_maps = [{"tet4": tet_p[c * NTPC_PAD:(c + 1) * NTPC_PAD],
                "occ4": occ4_p[c * NTPC_PAD:(c + 1) * NTPC_PAD]}
               for c in range(NCORES)]
    res_a = _run(nc_a, in_maps, "tet")
    ea6 = np.concatenate([r["ea6"] for r in res_a], 0)[:NT]
    eb6 = np.concatenate([r["eb6"] for r in res_a], 0)[:NT]
    ti = np.concatenate([r["tindex"] for r in res_a], 0)[:NT]

    # ---- host: valid filter + sort/dedup + rank mapping ----
    valid = (ti > 0) & (ti < 15)
    a = ea6[valid].reshape(-1).astype(np.int64)
    b = eb6[valid].reshape(-1).astype(np.int64)
    key = a * NV + b
    n = key.shape[0]
    # pack the slot index into the low 23 bits (n < 2^23, key < 2^36) so a
    # single in-place sort yields both sorted keys and the permutation
    packed = (key << 23) | np.arange(n, dtype=np.int64)
    packed.sort()
    sk = packed >> 23
    order = packed & ((1 << 23) - 1)
    new = np.empty(n, np.bool_)
    new[0] = True
    np.not_equal(sk[1:], sk[:-1], out=new[1:])
    uid = np.cumsum(new) - 1                 # rank among uniques, sorted order
    inv = np.empty(n, np.int64)
    inv[order] = uid
    uk = sk[new]
    ua = (uk // NV).astype(np.int32)
    ub = (uk - (uk // NV) * NV).astype(np.int32)
    cross = occ[ua] != occ[ub]
    mapping = np.where(cross, np.cumsum(cross, dtype=np.int64) - 1,
                       -1).astype(np.int32)
    idx_map = mapping[inv].reshape(-1, 6)
    ia = ua[cross]
    ib = ub[cross]                           # unique crossing edges, lex order
    ne = ia.shape[0]

    # ---- device B: interpolation + uv grid ----
    per = -(-ne // NCORES)                   # ceil
    ek = -(-per // P)
    ek = max(ek, 1)
    epc = P * ek
    if ("interp", ek) not in _prog_cache:
        _prog_cache[("interp", ek)] = _build_interp_program(ek)
    nc_b = _prog_cache[("interp", ek)]

    posT = np.empty((4, NV), np.float32)                 # SoA planes
    posT[:3] = pos.T
    posT[3] = sdf
    ia_p = np.zeros(epc * NCORES, np.int32)
    ib_p = np.zeros(epc * NCORES, np.int32)
    ia_p[:ne] = ia
    ib_p[:ne] = ib
    recA = posT[:, ia_p]                                 # [4, 8*epc]
    recB = posT[:, ib_p]

    lin = np.linspace(0.0, 1.0 - 1.0 / UVN, UVN, dtype=np.float32)
    lin_bcast = np.ascontiguousarray(np.broadcast_to(lin, (P, UVN)))
    lini_all = np.zeros(NCORES * P, np.float32)
    lini_all[:UVN] = lin
    in_maps = []
    for c in range(NCORES):
        in_maps.append({
            "recA": np.ascontiguousarray(recA[:, c * epc:(c + 1) * epc]),
            "recB": np.ascontiguousarray(recB[:, c * epc:(c + 1) * epc]),
            "linj": lin_bcast,
            "lini": lini_all[c * P:(c + 1) * P],
        })
    res_b = _run(nc_b, in_maps, "interp")
    verts = np.ascontiguousarray(
        np.concatenate([r["verts"] for r in res_b], 1)[:, :ne].T)
    uvrows = np.concatenate([r["uvblk"] for r in res_b], 0)  # [8*128, UVN, 8]
    uvs = uvrows[:UVN].reshape(UVN * UVN * 4, 2)

    # ---- host: faces + uv_idx ----
    ti_v = ti[valid]
    ntri = NUM_TRI_TABLE[ti_v]
    m1 = ntri == 1
    m2 = ntri == 2
    f1 = np.take_along_axis(idx_map[m1], TRIANGLE_TABLE[ti_v[m1]][:, :3],
                            axis=1).reshape(-1, 3)
    f2 = np.take_along_axis(idx_map[m2], TRIANGLE_TABLE[ti_v[m2]][:, :6],
                            axis=1).reshape(-1, 3)
    faces = np.concatenate([f1, f2], 0).astype(np.int32)

    tg = np.arange(NT, dtype=np.int32)[valid]
    fg = np.concatenate([tg[m1] * 2,
                         np.stack([tg[m2] * 2, tg[m2] * 2 + 1],
                                  axis=-1).reshape(-1)], 0)
    t_ = fg // 2
    r_ = fg % 2
    uv_idx = np.stack([t_ * 4, t_ * 4 + r_ + 1, t_ * 4 + r_ + 2],
                      axis=-1).astype(np.int32)

    return verts, faces, uvs, uv_idx
